# revision 1
# baseline (speedup 1.0000x reference)
"""Trainium2 Bass kernel for nn_Attention_9242769622327.

Math: the reference computes
    qkv = x @ W1.T ; q,k,v = split(qkv)
    score = softmax(k^T v / 4, axis=-1)            # rows sum to 1
    attn  = softmax(einsum('bhnk,bhkc->bhnk', q/4, score), axis=-1)
          = softmax(q/4 * sum_c score)             # sum_c score == 1
          = softmax(q/4)                           # k/v are mathematically dead
    out   = attn @ W2.T
so only the q-projection (first E rows of W1), a per-head (64-wide) softmax,
and the output projection are needed.

Distribution: pure data-parallel over the 32768 = B*S rows; each of the 8
cores handles 4096 rows with the full (transposed, fp16) weights. No
collectives.  fp16 runs the PE at the same 1 cycle/row as bf16 but with a
10-bit mantissa (rel err ~4.5e-4 vs ~3.6e-3 for bf16).

On-chip layout is fully transposed (features on partitions, rows on the free
dim) so no on-chip transposes are needed anywhere:
    qT[n,m]  = sum_k W1qT[k,n] * xT[k,m]          (PE, fp16)
    u        = exp(qT/4)                          (ACT, PSUM->SBUF fp16)
    s[g,m]   = sum_{n in head g} u[n,m]           (PE matmul w/ 0/1 selector)
    rcp      = 1/s                                (DVE reciprocal_approx_fast;
                                                   no Ln -> one ACT table set)
    rb[n,m]  = rcp[head(n),m]                     (PE matmul w/ selector^T,
                                                   K padded to 128 so LDW overlaps)
    aT       = u * rb                             (DVE)
    outT[j,m]= sum_n W2T[n,j] * aT[n,m]           (PE, fp16)

Stripes are software-pipelined: stripe ms runs [64 mm1][8 rb(ms-1)][8 sel]
[64 mm2(ms-1)] as contiguous same-shape matmul blocks on the PE (back-to-back
216ns issue at N=512), with exp/reciprocal/normalize hidden underneath.
Measured: 280.8us on 8 cores, rel err 4.5e-4 (vs ~249us pure-matmul floor).
"""

import sys

sys.path.insert(0, "/opt/trn_rl_repo")

import numpy as np
import ml_dtypes

import concourse.bass as bass
import concourse.bacc as bacc
import concourse.tile as tile
from concourse import mybir
from concourse.bass_utils import run_bass_kernel_spmd

BF16 = mybir.dt.float16  # fp16: same PE rate as bf16, 10-bit mantissa
F32 = mybir.dt.float32
AF = mybir.ActivationFunctionType

N_CORES = 8
B, S, E = 4, 8192, 1024
HEADS, HEAD_DIM = 16, 64
M_TOTAL = B * S                # 32768
M_CORE = M_TOTAL // N_CORES    # 4096 rows per core
MS = 512                       # m-stripe width (moving free dim / PSUM bank)
N_STRIPES = M_CORE // MS       # 8
KC = E // 128                  # 8 contraction chunks
NC_ = E // 128                 # 8 feature chunks

_BF = np.float16


def build_nc() -> bass.Bass:
    nc = bacc.Bacc("TRN2", debug=False)

    xt = nc.dram_tensor("xt", [E, M_CORE], BF16, kind="ExternalInput")
    w1t = nc.dram_tensor("w1t", [E, E], BF16, kind="ExternalInput")
    w2t = nc.dram_tensor("w2t", [E, E], BF16, kind="ExternalInput")
    sel = nc.dram_tensor("sel", [128, NC_ * HEADS], BF16, kind="ExternalInput")
    selt = nc.dram_tensor("selt", [128, NC_ * 128], BF16, kind="ExternalInput")
    outT = nc.dram_tensor("outT", [E, M_CORE], BF16, kind="ExternalOutput")

    xt_v = xt[:, :].rearrange("(c p) m -> p c m", p=128)    # [128, 8, M_CORE]
    w1_v = w1t[:, :].rearrange("(c p) n -> p c n", p=128)   # [128, 8, 1024]
    w2_v = w2t[:, :].rearrange("(c p) j -> p c j", p=128)   # [128, 8, 1024]

    with tile.TileContext(nc) as tc:
        with (
            tc.tile_pool(name="weights", bufs=1) as wpool,
            tc.tile_pool(name="xt", bufs=N_STRIPES) as xpool,
            tc.tile_pool(name="u", bufs=16) as upool,
            tc.tile_pool(name="at", bufs=16) as apool,
            tc.tile_pool(name="small", bufs=3) as spool,
            tc.tile_pool(name="ostage", bufs=8) as opool,
            tc.tile_pool(name="ps_q", bufs=2, space="PSUM") as psq,
            tc.tile_pool(name="ps_s", bufs=2, space="PSUM") as pss,
            tc.tile_pool(name="ps_rb", bufs=2, space="PSUM") as psrb,
            tc.tile_pool(name="ps_o", bufs=2, space="PSUM") as pso,
        ):
            # Per-chunk weight tiles so the first matmuls only wait on the
            # chunks they read, not the whole 4MB of weights.  Load order:
            # w1 + sel (needed by stripe 0's mm1/sel), stripe-0 x chunks,
            # then w2 + selt (not needed until ~18us in).
            # Warm the PE's HAM clock gate with throwaway matmuls on memset
            # scratch while the first weight/x DMAs are in flight, so the
            # first real matmuls run at 2.4 GHz instead of 1.2.
            warm_sb = wpool.tile([128, MS], BF16, name="warm_sb")
            nc.gpsimd.memset(warm_sb[:], 0.0)
            warm_ps = psq.tile([128, MS], F32, tag="q", name="warm_ps")
            for _ in range(16):
                nc.tensor.matmul(
                    warm_ps[:], warm_sb[:, 0:128], warm_sb[:], start=True, stop=True
                )

            w1_k = []
            xt0 = []
            for kc in range(KC):
                t = wpool.tile([128, E], BF16, tag=f"w1_{kc}", name=f"w1k{kc}")
                nc.sync.dma_start(t[:], w1_v[:, kc, :])
                w1_k.append(t)
                tx = xpool.tile([128, MS], BF16, tag=f"xt_{kc}", name=f"xt0_{kc}")
                nc.sync.dma_start(tx[:], xt_v[:, kc, 0:MS])
                xt0.append(tx)
            sel_t = wpool.tile([128, NC_, HEADS], BF16, name="sel_t")
            nc.sync.dma_start(sel_t[:], sel[:, :].rearrange("p (c g) -> p c g", g=HEADS))

            w2_k = []
            for ci in range(NC_):
                t = wpool.tile([128, E], BF16, tag=f"w2_{ci}", name=f"w2k{ci}")
                nc.sync.dma_start(t[:], w2_v[:, ci, :])
                w2_k.append(t)
            selt_t = wpool.tile([128, NC_, 128], BF16, name="selt_t")
            nc.sync.dma_start(selt_t[:], selt[:, :].rearrange("p (c q) -> p c q", q=128))

            # Software pipeline over stripes: while stripe ms runs its
            # q-projection (mm1) + exp + head-sum on the PE, stripe ms-1's
            # normalization (rb broadcast matmul + DVE mul) and output
            # projection (mm2) are interleaved so the PE never waits on the
            # softmax chain.
            prev_u = None       # u tiles of stripe ms-1
            prev_rcp = None     # reciprocal head-sums of stripe ms-1 (bf16)
            prev_ms = -1

            def emit_norm(pu, prcp):
                """rb broadcast matmuls (PE, contiguous block, K padded to 128
                so LDWEIGHTS overlaps like the main GEMM blocks) + DVE muls."""
                ats = []
                for ci in range(NC_):
                    rb_ps = psrb.tile([128, MS], F32, tag="rb", name="rb_ps")
                    nc.tensor.matmul(
                        rb_ps[:], selt_t[:, ci, :], prcp[:], start=True, stop=True
                    )
                    at_t = apool.tile([128, MS], BF16, tag="at", name="at_t")
                    nc.vector.tensor_mul(at_t[:], pu[ci][:], rb_ps[:])
                    ats.append(at_t)
                return ats

            def emit_tail(at_list, ms):
                """Emit mm2 + store for a finished stripe (at tiles ready)."""
                for j in range(NC_):
                    o_ps = pso.tile([128, MS], F32, tag="o", name="o_ps")
                    for ci in range(NC_):
                        nc.tensor.matmul(
                            o_ps[:],
                            w2_k[ci][:, j * 128:(j + 1) * 128],
                            at_list[ci][:],
                            start=(ci == 0),
                            stop=(ci == NC_ - 1),
                        )
                    o_t = opool.tile([128, MS], BF16, tag="ost", name="o_t")
                    nc.scalar.copy(o_t[:], o_ps[:])
                    nc.sync.dma_start(
                        outT[j * 128:(j + 1) * 128, ms * MS:(ms + 1) * MS], o_t[:]
                    )

            for ms in range(N_STRIPES):
                if ms == 0:
                    xt_k = xt0
                else:
                    xt_k = []
                    for kc in range(KC):
                        t = xpool.tile(
                            [128, MS], BF16, tag=f"xt_{kc}", name=f"xt{ms}_{kc}"
                        )
                        nc.sync.dma_start(
                            t[:], xt_v[:, kc, ms * MS:(ms + 1) * MS]
                        )
                        xt_k.append(t)

                # ---- mm1: q-projection, contiguous 64-MM block on PE ----
                u_tiles = []
                q_list = []
                for ci in range(NC_):
                    q_ps = psq.tile([128, MS], F32, tag="q", name="q_ps")
                    for kc in range(KC):
                        nc.tensor.matmul(
                            q_ps[:],
                            w1_k[kc][:, ci * 128:(ci + 1) * 128],
                            xt_k[kc][:],
                            start=(kc == 0),
                            stop=(kc == KC - 1),
                        )
                    u_t = upool.tile([128, MS], BF16, tag="u", name="u_t")
                    nc.scalar.activation(u_t[:], q_ps[:], AF.Exp, scale=0.25)
                    u_tiles.append(u_t)

                # ---- stripe ms-1 normalization (hides exp latency) ----
                at_tiles = emit_norm(prev_u, prev_rcp) if prev_rcp is not None else None

                # ---- head sums (contiguous 8-MM block) + reciprocal ----
                s_ps = pss.tile([HEADS, MS], F32, tag="s", name="s_ps")
                for ci in range(NC_):
                    nc.tensor.matmul(
                        s_ps[:],
                        sel_t[:, ci, :],
                        u_tiles[ci][:],
                        start=(ci == 0),
                        stop=(ci == NC_ - 1),
                    )
                rcp32 = spool.tile([HEADS, MS], F32, tag="rcp32", name="rcp32")
                nc.vector.reciprocal_approx_fast(rcp32[:], s_ps[:])
                # rcp padded to 128 partitions (rows 16+ zeroed on the idle
                # GpSimd engine) so the rb matmul runs with K=128
                rcp_t = spool.tile([128, MS], BF16, tag="rcp", name="rcp_t")
                nc.gpsimd.memset(rcp_t[:], 0.0)
                nc.scalar.copy(rcp_t[0:HEADS, :], rcp32[:])

                # ---- stripe ms-1 output projection ----
                if at_tiles is not None:
                    emit_tail(at_tiles, prev_ms)
                prev_u, prev_rcp, prev_ms = u_tiles, rcp_t, ms

            # epilogue: last stripe's normalization + output projection
            at_tiles = emit_norm(prev_u, prev_rcp)
            emit_tail(at_tiles, prev_ms)
    nc.compile()
    return nc


_NC_CACHE = None
LAST_RESULT = None


def _ensure_ntff_hook():
    """bass_utils' axon trace path needs antenv.axon_hooks, which this
    container's antenv lacks. Provide it + register the ctypes NTFF hook."""
    import types

    try:
        from antenv.axon_hooks import get_axon_ntff_profile_hook  # noqa: F401
        return True
    except ImportError:
        pass
    try:
        import antenv
        from trn_agent_boot.trn_boot import _ntff_profile_via_ctypes

        m = types.ModuleType("antenv.axon_hooks")
        state = {"hook": None}
        m.set_axon_ntff_profile_hook = lambda h: state.__setitem__("hook", h)
        m.get_axon_ntff_profile_hook = lambda: state["hook"]
        sys.modules["antenv.axon_hooks"] = m
        antenv.axon_hooks = m
        m.set_axon_ntff_profile_hook(
            _ntff_profile_via_ctypes("/opt/axon/libaxon_pjrt.so")
        )
        return True
    except Exception as e:  # pragma: no cover
        print(f"ntff hook injection failed: {e}")
        return False


def _selectors():
    # head index of global feature n is n // 64; chunk ci covers n in
    # [128ci, 128ci+128) -> heads 2ci (partitions 0..63) and 2ci+1 (64..127)
    sel = np.zeros((128, NC_, HEADS), np.float32)
    selt = np.zeros((128, NC_, 128), np.float32)  # K padded to 128, rows 16+ zero
    for ci in range(NC_):
        sel[:64, ci, 2 * ci] = 1.0
        sel[64:, ci, 2 * ci + 1] = 1.0
        selt[2 * ci, ci, :64] = 1.0
        selt[2 * ci + 1, ci, 64:] = 1.0
    return (
        np.ascontiguousarray(sel.reshape(128, NC_ * HEADS)).astype(_BF),
        np.ascontiguousarray(selt.reshape(128, NC_ * 128)).astype(_BF),
    )


def kernel(x, W1, W2, heads, trace=False):
    global _NC_CACHE, LAST_RESULT
    x = np.asarray(x, dtype=np.float32)
    W1 = np.asarray(W1, dtype=np.float32)
    W2 = np.asarray(W2, dtype=np.float32)

    X = x.reshape(M_TOTAL, E)
    Xbf = X.astype(_BF)
    XbfT = Xbf.T  # [E, M_TOTAL] view
    w1t = np.ascontiguousarray(W1[:E, :].T).astype(_BF)   # [k, n] = W1q[n, k]
    w2t = np.ascontiguousarray(W2.T).astype(_BF)          # [n, j] = W2[j, n]
    sel, selt = _selectors()

    in_maps = []
    for c in range(N_CORES):
        xt_c = np.ascontiguousarray(XbfT[:, c * M_CORE:(c + 1) * M_CORE])
        in_maps.append(
            {"xt": xt_c, "w1t": w1t, "w2t": w2t, "sel": sel, "selt": selt}
        )

    if _NC_CACHE is None:
        _NC_CACHE = build_nc()

    if trace:
        trace = _ensure_ntff_hook()

    res = run_bass_kernel_spmd(_NC_CACHE, in_maps, list(range(N_CORES)), trace=trace)
    LAST_RESULT = res

    OT = np.concatenate(
        [np.asarray(res.results[c]["outT"]).astype(np.float32) for c in range(N_CORES)],
        axis=1,
    )
    return np.ascontiguousarray(OT.T).reshape(B, S, E)



# revision 2
# speedup vs baseline: 1.2626x; 1.2626x over previous
"""Trainium2 Bass kernel for nn_Attention_9242769622327.

Math: the reference computes
    qkv = x @ W1.T ; q,k,v = split(qkv)
    score = softmax(k^T v / 4, axis=-1)            # rows sum to 1
    attn  = softmax(einsum('bhnk,bhkc->bhnk', q/4, score), axis=-1)
          = softmax(q/4 * sum_c score)             # sum_c score == 1
          = softmax(q/4)                           # k/v are mathematically dead
    out   = attn @ W2.T
so only the q-projection (first E rows of W1), a per-head (64-wide) softmax,
and the output projection are needed.

Distribution: pure data-parallel over the 32768 = B*S rows; each of the 8
cores handles 4096 rows with the full weights. No collectives.

Precision: mm1 (q-projection) runs in fp8-e4m3 DoubleRow (2 fp8 MACs per PE
cell per cycle -> half the matmul instructions of bf16).  Its ~2.5%
quantization noise on q is attenuated by the /4 + exp + softmax chain to
~0.9% on the output.  mm2 must stay fp16: quantizing attn to fp8 alone costs
~2.5% on the output (threshold is 2%).  W1q is pre-scaled by 32 so its
entries (std 1/32) use the fp8 dynamic range; the exp() activation applies
scale 1/(4*32) to compensate.

On-chip layout is fully transposed (features on partitions, rows on the free
dim) so no on-chip transposes are needed anywhere:
    qT[n,m]  = sum_k W1qT[k,n] * xT[k,m]          (PE, fp8 DoubleRow K=256/MM)
    u        = exp(qT/128)                        (ACT, PSUM->SBUF fp16)
    s[g,m]   = sum_{n in head g} u[n,m]           (PE matmul w/ 0/1 selector)
    rcp      = 1/s                                (DVE reciprocal_approx_fast)
    rb[n,m]  = rcp[head(n),m]                     (PE matmul w/ selector^T)
    aT       = u * rb                             (DVE)
    outT[j,m]= sum_n W2T[n,j] * aT[n,m]           (PE, fp16)

Stripes are software-pipelined: stripe ms runs [32 mm1-DR][8 rb(ms-1)][8 sel]
[64 mm2(ms-1)] as contiguous matmul blocks on the PE, with exp/reciprocal/
normalize hidden underneath.
"""

import sys

sys.path.insert(0, "/opt/trn_rl_repo")

import numpy as np
import ml_dtypes

import concourse.bass as bass
import concourse.bacc as bacc
import concourse.tile as tile
from concourse import mybir
from concourse.bass_utils import run_bass_kernel_spmd

BF16 = mybir.dt.float16  # fp16: same PE rate as bf16, 10-bit mantissa
FP8 = mybir.dt.float8e4
F32 = mybir.dt.float32
AF = mybir.ActivationFunctionType
DR = mybir.MatmulPerfMode.DoubleRow

N_CORES = 8
B, S, E = 4, 8192, 1024
HEADS, HEAD_DIM = 16, 64
M_TOTAL = B * S                # 32768
M_CORE = M_TOTAL // N_CORES    # 4096 rows per core
MS = 512                       # m-stripe width (moving free dim / PSUM bank)
N_STRIPES = M_CORE // MS       # 8
KC = E // 128                  # 8 contraction chunks (bf16 view)
KP = E // 256                  # 4 DoubleRow contraction pair-chunks
NC_ = E // 128                 # 8 feature chunks
W1_SCALE = 32.0                # pre-scale on W1q before fp8 quantization

_BF = np.float16
_F8 = ml_dtypes.float8_e4m3fn


def build_nc() -> bass.Bass:
    nc = bacc.Bacc("TRN2", debug=False)

    xt8 = nc.dram_tensor("xt8", [E, M_CORE], FP8, kind="ExternalInput")
    w18 = nc.dram_tensor("w18", [E, E], FP8, kind="ExternalInput")
    w2t = nc.dram_tensor("w2t", [E, E], BF16, kind="ExternalInput")
    sel = nc.dram_tensor("sel", [128, NC_ * HEADS], BF16, kind="ExternalInput")
    selt = nc.dram_tensor("selt", [128, NC_ * 128], BF16, kind="ExternalInput")
    outT = nc.dram_tensor("outT", [E, M_CORE], BF16, kind="ExternalOutput")

    # k = t*256 + two*128 + p
    x8_v = xt8[:, :].rearrange("(t two p) m -> p t two m", p=128, two=2)
    w18_v = w18[:, :].rearrange("(t two p) n -> p t two n", p=128, two=2)
    w2_v = w2t[:, :].rearrange("(c p) j -> p c j", p=128)   # [128, 8, 1024]

    with tile.TileContext(nc) as tc:
        with (
            tc.tile_pool(name="weights", bufs=1) as wpool,
            tc.tile_pool(name="xt", bufs=N_STRIPES) as xpool,
            tc.tile_pool(name="u", bufs=16) as upool,
            tc.tile_pool(name="at", bufs=16) as apool,
            tc.tile_pool(name="small", bufs=3) as spool,
            tc.tile_pool(name="ostage", bufs=8) as opool,
            tc.tile_pool(name="ps_q", bufs=2, space="PSUM") as psq,
            tc.tile_pool(name="ps_s", bufs=2, space="PSUM") as pss,
            tc.tile_pool(name="ps_rb", bufs=2, space="PSUM") as psrb,
            tc.tile_pool(name="ps_o", bufs=2, space="PSUM") as pso,
        ):
            # Warm the PE's HAM clock gate with throwaway matmuls on memset
            # scratch while the first weight/x DMAs are in flight, so the
            # first real matmuls run at 2.4 GHz instead of 1.2.
            warm_sb = wpool.tile([128, MS], BF16, name="warm_sb")
            nc.gpsimd.memset(warm_sb[:], 0.0)
            warm_ps = psq.tile([128, MS], F32, tag="q", name="warm_ps")
            for _ in range(16):
                nc.tensor.matmul(
                    warm_ps[:], warm_sb[:, 0:128], warm_sb[:], start=True, stop=True
                )

            # Per-pair-chunk weight tiles so the first matmuls only wait on
            # the chunks they read.  Load order: w1 + sel (needed by stripe
            # 0's mm1/sel), stripe-0 x chunks, then w2 + selt.
            w1_t = []
            xt0 = []
            for t in range(KP):
                w = wpool.tile([128, 2, E], FP8, tag=f"w1_{t}", name=f"w1t{t}")
                nc.sync.dma_start(w[:], w18_v[:, t, :, :])
                w1_t.append(w)
                tx = xpool.tile([128, 2, MS], FP8, tag=f"xt_{t}", name=f"xt0_{t}")
                nc.sync.dma_start(tx[:], x8_v[:, t, :, 0:MS])
                xt0.append(tx)
            sel_t = wpool.tile([128, NC_, HEADS], BF16, name="sel_t")
            nc.sync.dma_start(sel_t[:], sel[:, :].rearrange("p (c g) -> p c g", g=HEADS))

            w2_k = []
            for ci in range(NC_):
                t = wpool.tile([128, E], BF16, tag=f"w2_{ci}", name=f"w2k{ci}")
                nc.sync.dma_start(t[:], w2_v[:, ci, :])
                w2_k.append(t)
            selt_t = wpool.tile([128, NC_, 128], BF16, name="selt_t")
            nc.sync.dma_start(selt_t[:], selt[:, :].rearrange("p (c q) -> p c q", q=128))

            # Software pipeline over stripes: while stripe ms runs its
            # q-projection (mm1) + exp + head-sum on the PE, stripe ms-1's
            # normalization (rb broadcast matmul + DVE mul) and output
            # projection (mm2) are interleaved so the PE never waits on the
            # softmax chain.
            prev_u = None       # u tiles of stripe ms-1
            prev_rcp = None     # reciprocal head-sums of stripe ms-1 (fp16)
            prev_ms = -1

            def emit_norm(pu, prcp):
                """rb broadcast matmuls (PE, contiguous block, K padded to 128
                so LDWEIGHTS overlaps like the main GEMM blocks) + DVE muls."""
                ats = []
                for ci in range(NC_):
                    rb_ps = psrb.tile([128, MS], F32, tag="rb", name="rb_ps")
                    nc.tensor.matmul(
                        rb_ps[:], selt_t[:, ci, :], prcp[:], start=True, stop=True
                    )
                    at_t = apool.tile([128, MS], BF16, tag="at", name="at_t")
                    nc.vector.tensor_mul(at_t[:], pu[ci][:], rb_ps[:])
                    ats.append(at_t)
                return ats

            def emit_tail(at_list, ms):
                """Emit mm2 + store for a finished stripe (at tiles ready)."""
                for j in range(NC_):
                    o_ps = pso.tile([128, MS], F32, tag="o", name="o_ps")
                    for ci in range(NC_):
                        nc.tensor.matmul(
                            o_ps[:],
                            w2_k[ci][:, j * 128:(j + 1) * 128],
                            at_list[ci][:],
                            start=(ci == 0),
                            stop=(ci == NC_ - 1),
                        )
                    o_t = opool.tile([128, MS], BF16, tag="ost", name="o_t")
                    nc.scalar.copy(o_t[:], o_ps[:])
                    nc.sync.dma_start(
                        outT[j * 128:(j + 1) * 128, ms * MS:(ms + 1) * MS], o_t[:]
                    )

            for ms in range(N_STRIPES):
                if ms == 0:
                    xt_k = xt0
                else:
                    xt_k = []
                    for t in range(KP):
                        tx = xpool.tile(
                            [128, 2, MS], FP8, tag=f"xt_{t}", name=f"xt{ms}_{t}"
                        )
                        nc.sync.dma_start(
                            tx[:], x8_v[:, t, :, ms * MS:(ms + 1) * MS]
                        )
                        xt_k.append(tx)

                # ---- mm1: q-projection, contiguous 32-MM fp8-DR block ----
                u_tiles = []
                for ci in range(NC_):
                    q_ps = psq.tile([128, MS], F32, tag="q", name="q_ps")
                    for t in range(KP):
                        nc.tensor.matmul(
                            q_ps[:],
                            w1_t[t][:, :, ci * 128:(ci + 1) * 128],
                            xt_k[t][:],
                            start=(t == 0),
                            stop=(t == KP - 1),
                            perf_mode=DR,
                        )
                    u_t = upool.tile([128, MS], BF16, tag="u", name="u_t")
                    nc.scalar.activation(
                        u_t[:], q_ps[:], AF.Exp, scale=1.0 / (4.0 * W1_SCALE)
                    )
                    u_tiles.append(u_t)

                # ---- stripe ms-1 normalization (hides exp latency) ----
                at_tiles = emit_norm(prev_u, prev_rcp) if prev_rcp is not None else None

                # ---- head sums (contiguous 8-MM block) + reciprocal ----
                s_ps = pss.tile([HEADS, MS], F32, tag="s", name="s_ps")
                for ci in range(NC_):
                    nc.tensor.matmul(
                        s_ps[:],
                        sel_t[:, ci, :],
                        u_tiles[ci][:],
                        start=(ci == 0),
                        stop=(ci == NC_ - 1),
                    )
                rcp32 = spool.tile([HEADS, MS], F32, tag="rcp32", name="rcp32")
                nc.vector.reciprocal_approx_fast(rcp32[:], s_ps[:])
                # rcp padded to 128 partitions (rows 16+ zeroed on the idle
                # GpSimd engine) so the rb matmul runs with K=128
                rcp_t = spool.tile([128, MS], BF16, tag="rcp", name="rcp_t")
                nc.gpsimd.memset(rcp_t[:], 0.0)
                nc.scalar.copy(rcp_t[0:HEADS, :], rcp32[:])

                # ---- stripe ms-1 output projection ----
                if at_tiles is not None:
                    emit_tail(at_tiles, prev_ms)
                prev_u, prev_rcp, prev_ms = u_tiles, rcp_t, ms

            # epilogue: last stripe's normalization + output projection
            at_tiles = emit_norm(prev_u, prev_rcp)
            emit_tail(at_tiles, prev_ms)
    nc.compile()
    return nc


_NC_CACHE = None
LAST_RESULT = None


def _ensure_ntff_hook():
    """bass_utils' axon trace path needs antenv.axon_hooks, which this
    container's antenv lacks. Provide it + register the ctypes NTFF hook."""
    import types

    try:
        from antenv.axon_hooks import get_axon_ntff_profile_hook  # noqa: F401
        return True
    except ImportError:
        pass
    try:
        import antenv
        from trn_agent_boot.trn_boot import _ntff_profile_via_ctypes

        m = types.ModuleType("antenv.axon_hooks")
        state = {"hook": None}
        m.set_axon_ntff_profile_hook = lambda h: state.__setitem__("hook", h)
        m.get_axon_ntff_profile_hook = lambda: state["hook"]
        sys.modules["antenv.axon_hooks"] = m
        antenv.axon_hooks = m
        m.set_axon_ntff_profile_hook(
            _ntff_profile_via_ctypes("/opt/axon/libaxon_pjrt.so")
        )
        return True
    except Exception as e:  # pragma: no cover
        print(f"ntff hook injection failed: {e}")
        return False


def _selectors():
    # head index of global feature n is n // 64; chunk ci covers n in
    # [128ci, 128ci+128) -> heads 2ci (partitions 0..63) and 2ci+1 (64..127)
    sel = np.zeros((128, NC_, HEADS), np.float32)
    selt = np.zeros((128, NC_, 128), np.float32)  # K padded to 128, rows 16+ zero
    for ci in range(NC_):
        sel[:64, ci, 2 * ci] = 1.0
        sel[64:, ci, 2 * ci + 1] = 1.0
        selt[2 * ci, ci, :64] = 1.0
        selt[2 * ci + 1, ci, 64:] = 1.0
    return (
        np.ascontiguousarray(sel.reshape(128, NC_ * HEADS)).astype(_BF),
        np.ascontiguousarray(selt.reshape(128, NC_ * 128)).astype(_BF),
    )


def kernel(x, W1, W2, heads, trace=False):
    global _NC_CACHE, LAST_RESULT
    x = np.asarray(x, dtype=np.float32)
    W1 = np.asarray(W1, dtype=np.float32)
    W2 = np.asarray(W2, dtype=np.float32)

    X = x.reshape(M_TOTAL, E)
    X8T = np.ascontiguousarray(X.T).astype(_F8)           # [E, M_TOTAL]
    w18 = np.ascontiguousarray((W1[:E, :] * W1_SCALE).T).astype(_F8)  # [k, n]
    w2t = np.ascontiguousarray(W2.T).astype(_BF)          # [n, j] = W2[j, n]
    sel, selt = _selectors()

    in_maps = []
    for c in range(N_CORES):
        xt_c = np.ascontiguousarray(X8T[:, c * M_CORE:(c + 1) * M_CORE])
        in_maps.append(
            {"xt8": xt_c, "w18": w18, "w2t": w2t, "sel": sel, "selt": selt}
        )

    if _NC_CACHE is None:
        _NC_CACHE = build_nc()

    if trace:
        trace = _ensure_ntff_hook()

    res = run_bass_kernel_spmd(_NC_CACHE, in_maps, list(range(N_CORES)), trace=trace)
    LAST_RESULT = res

    OT = np.concatenate(
        [np.asarray(res.results[c]["outT"]).astype(np.float32) for c in range(N_CORES)],
        axis=1,
    )
    return np.ascontiguousarray(OT.T).reshape(B, S, E)


# revision 3
# speedup vs baseline: 1.3445x; 1.0648x over previous
"""Trainium2 Bass kernel for nn_Attention_9242769622327.

Math: the reference computes
    qkv = x @ W1.T ; q,k,v = split(qkv)
    score = softmax(k^T v / 4, axis=-1)            # rows sum to 1
    attn  = softmax(einsum('bhnk,bhkc->bhnk', q/4, score), axis=-1)
          = softmax(q/4 * sum_c score)             # sum_c score == 1
          = softmax(q/4)                           # k/v are mathematically dead
    out   = attn @ W2.T
so only the q-projection (first E rows of W1), a per-head (64-wide) softmax,
and the output projection are needed.

Distribution: pure data-parallel over the 32768 = B*S rows; each of the 8
cores handles 4096 rows with the full weights. No collectives.

Precision: mm1 (q-projection) and the head-sum matmul run in fp8-e4m3
DoubleRow (2 fp8 MACs per PE cell per cycle -> half the matmul instructions
of bf16; measured same 213ns/MM issue rate at N=512).  The ~2.5% fp8 noise
on q is attenuated by the /4 + exp + softmax chain to ~1% on the output; the
head-sum noise is averaged down by the 64-wide sum.  mm2 must stay fp16:
quantizing attn to fp8 alone costs ~2.5% on the output (threshold is 2%).
W1q is pre-scaled by 32 so its entries (std 1/32) use the fp8 dynamic range;
the exp() activation applies scale 1/(4*32) to compensate.

On-chip layout is fully transposed (features on partitions, rows on the free
dim) so no on-chip transposes are needed anywhere:
    qT[n,m]  = sum_k W1qT[k,n] * xT[k,m]          (PE, fp8 DR, K=256/MM)
    u        = exp(qT/128)                        (ACT, PSUM->SBUF fp16)
    u8       = fp8(u)                             (DVE, pair-interleaved)
    s[g,m]   = sum_{n in head g} u8[n,m]          (PE fp8-DR w/ 0/1 selector)
    rcp      = 1/s                                (DVE reciprocal_approx_fast)
    rb[n,m]  = rcp[head(n),m]                     (PE matmul w/ selector^T)
    aT       = u * rb                             (DVE)
    outT[j,m]= sum_n W2T[n,j] * aT[n,m]           (PE, fp16)

Stripes are software-pipelined: stripe ms runs [32 mm1-DR][8 rb(ms-1)]
[64 mm2(ms-1) with the 4 sel-DR MMs of stripe ms pushed between the mm2
j-blocks] so the exp->u8 chain never stalls the PE.  PSUM: 3 q banks, 1 s,
2 rb, 2 out.  w2/selt DMAs are deferred behind stripe-0/1 x + w1 loads so
the first mm1 block starts as early as possible.
"""

import sys

sys.path.insert(0, "/opt/trn_rl_repo")

import numpy as np
import ml_dtypes

import concourse.bass as bass
import concourse.bacc as bacc
import concourse.tile as tile
from concourse import mybir
from concourse.bass_utils import run_bass_kernel_spmd

BF16 = mybir.dt.float16  # fp16: same PE rate as bf16, 10-bit mantissa
FP8 = mybir.dt.float8e4
F32 = mybir.dt.float32
AF = mybir.ActivationFunctionType
DR = mybir.MatmulPerfMode.DoubleRow

N_CORES = 8
B, S, E = 4, 8192, 1024
HEADS, HEAD_DIM = 16, 64
M_TOTAL = B * S                # 32768
M_CORE = M_TOTAL // N_CORES    # 4096 rows per core
MS = 512                       # m-stripe width (moving free dim / PSUM bank)
N_STRIPES = M_CORE // MS       # 8
KP = E // 256                  # 4 DoubleRow contraction pair-chunks
NC_ = E // 128                 # 8 feature chunks
W1_SCALE = 32.0                # pre-scale on W1q before fp8 quantization

_BF = np.float16
_F8 = ml_dtypes.float8_e4m3fn


def build_nc() -> bass.Bass:
    nc = bacc.Bacc("TRN2", debug=False)

    xt8 = nc.dram_tensor("xt8", [E, M_CORE], FP8, kind="ExternalInput")
    w18 = nc.dram_tensor("w18", [E, E], FP8, kind="ExternalInput")
    w2t = nc.dram_tensor("w2t", [E, E], BF16, kind="ExternalInput")
    sel8 = nc.dram_tensor("sel8", [128, KP * 2 * HEADS], FP8, kind="ExternalInput")
    selt = nc.dram_tensor("selt", [128, NC_ * 128], BF16, kind="ExternalInput")
    outT = nc.dram_tensor("outT", [E, M_CORE], BF16, kind="ExternalOutput")

    # k = t*256 + two*128 + p
    x8_v = xt8[:, :].rearrange("(t two p) m -> p t two m", p=128, two=2)
    w18_v = w18[:, :].rearrange("(t two p) n -> p t two n", p=128, two=2)
    w2_v = w2t[:, :].rearrange("(c p) j -> p c j", p=128)   # [128, 8, 1024]

    with tile.TileContext(nc) as tc:
        with (
            tc.tile_pool(name="weights", bufs=1) as wpool,
            tc.tile_pool(name="xt", bufs=N_STRIPES) as xpool,
            tc.tile_pool(name="u", bufs=16) as upool,
            tc.tile_pool(name="u8", bufs=2) as u8pool,
            tc.tile_pool(name="at", bufs=16) as apool,
            tc.tile_pool(name="small", bufs=3) as spool,
            tc.tile_pool(name="ostage", bufs=8) as opool,
            tc.tile_pool(name="ps_q", bufs=3, space="PSUM") as psq,
            tc.tile_pool(name="ps_s", bufs=1, space="PSUM") as pss,
            tc.tile_pool(name="ps_rb", bufs=2, space="PSUM") as psrb,
            tc.tile_pool(name="ps_o", bufs=2, space="PSUM") as pso,
        ):
            # Warm the PE's HAM clock gate with throwaway matmuls on memset
            # scratch while the first weight/x DMAs are in flight, so the
            # first real matmuls run at 2.4 GHz instead of 1.2.  ~10 cold MMs
            # x 427ns covers the ~3.4us HAM window.
            warm_sb = wpool.tile([128, MS], BF16, name="warm_sb")
            nc.gpsimd.memset(warm_sb[:], 0.0)
            warm_ps = psq.tile([128, MS], F32, tag="q", name="warm_ps")
            for _ in range(10):
                nc.tensor.matmul(
                    warm_ps[:], warm_sb[:, 0:128], warm_sb[:], start=True, stop=True
                )

            # Stripe-0-critical loads first: w1 pair-chunks interleaved with
            # stripe-0 x pair-chunks, then the tiny sel8.  w2/selt are
            # deferred until after stripe 1's x loads (they aren't read until
            # stripe 0's normalization/output projection, ~25us in).
            w1_t = []
            xt0 = []
            for t in range(KP):
                w = wpool.tile([128, 2, E], FP8, tag=f"w1_{t}", name=f"w1t{t}")
                nc.sync.dma_start(w[:], w18_v[:, t, :, :])
                w1_t.append(w)
                tx = xpool.tile([128, 2, MS], FP8, tag=f"xt_{t}", name=f"xt0_{t}")
                nc.sync.dma_start(tx[:], x8_v[:, t, :, 0:MS])
                xt0.append(tx)
            sel8_t = wpool.tile([128, KP, 2, HEADS], FP8, name="sel8_t")
            nc.sync.dma_start(
                sel8_t[:],
                sel8[:, :].rearrange("p (t two h) -> p t two h", two=2, h=HEADS),
            )

            w2_k = [
                wpool.tile([128, E], BF16, tag=f"w2_{ci}", name=f"w2k{ci}")
                for ci in range(NC_)
            ]
            selt_t = wpool.tile([128, NC_, 128], BF16, name="selt_t")

            # Software pipeline over stripes: while stripe ms runs its
            # q-projection (mm1) + exp + head-sum on the PE, stripe ms-1's
            # normalization (rb broadcast matmul + DVE mul) and output
            # projection (mm2) are interleaved so the PE never waits on the
            # softmax chain.
            prev_u = None       # u tiles of stripe ms-1
            prev_rcp = None     # reciprocal head-sums of stripe ms-1 (fp16)
            prev_ms = -1

            def emit_norm(pu, prcp):
                """rb broadcast matmuls (PE, K padded to 128 so LDWEIGHTS
                overlaps like the main GEMM blocks) + DVE muls."""
                ats = []
                for ci in range(NC_):
                    rb_ps = psrb.tile([128, MS], F32, tag="rb", name="rb_ps")
                    nc.tensor.matmul(
                        rb_ps[:], selt_t[:, ci, :], prcp[:], start=True, stop=True
                    )
                    at_t = apool.tile([128, MS], BF16, tag="at", name="at_t")
                    nc.vector.tensor_mul(at_t[:], pu[ci][:], rb_ps[:])
                    ats.append(at_t)
                return ats

            def emit_tail(at_list, ms, js):
                """Emit mm2 + store for a finished stripe (at tiles ready)."""
                for j in js:
                    o_ps = pso.tile([128, MS], F32, tag="o", name="o_ps")
                    for ci in range(NC_):
                        nc.tensor.matmul(
                            o_ps[:],
                            w2_k[ci][:, j * 128:(j + 1) * 128],
                            at_list[ci][:],
                            start=(ci == 0),
                            stop=(ci == NC_ - 1),
                        )
                    o_t = opool.tile([128, MS], BF16, tag="ost", name="o_t")
                    nc.scalar.copy(o_t[:], o_ps[:])
                    nc.sync.dma_start(
                        outT[j * 128:(j + 1) * 128, ms * MS:(ms + 1) * MS], o_t[:]
                    )

            for ms in range(N_STRIPES):
                if ms == 0:
                    xt_k = xt0
                else:
                    xt_k = []
                    for t in range(KP):
                        tx = xpool.tile(
                            [128, 2, MS], FP8, tag=f"xt_{t}", name=f"xt{ms}_{t}"
                        )
                        nc.sync.dma_start(
                            tx[:], x8_v[:, t, :, ms * MS:(ms + 1) * MS]
                        )
                        xt_k.append(tx)
                if ms == 1:
                    # deferred bulk loads (needed from stripe-0 norm onwards)
                    nc.sync.dma_start(
                        selt_t[:],
                        selt[:, :].rearrange("p (c q) -> p c q", q=128),
                    )
                    for ci in range(NC_):
                        nc.sync.dma_start(w2_k[ci][:], w2_v[:, ci, :])

                # ---- mm1: q-projection, contiguous 32-MM fp8-DR block ----
                u_tiles = []
                u8_tiles = []
                for ci in range(NC_):
                    q_ps = psq.tile([128, MS], F32, tag="q", name="q_ps")
                    for t in range(KP):
                        nc.tensor.matmul(
                            q_ps[:],
                            w1_t[t][:, :, ci * 128:(ci + 1) * 128],
                            xt_k[t][:],
                            start=(t == 0),
                            stop=(t == KP - 1),
                            perf_mode=DR,
                        )
                    u_t = upool.tile([128, MS], BF16, tag="u", name="u_t")
                    nc.scalar.activation(
                        u_t[:], q_ps[:], AF.Exp, scale=1.0 / (4.0 * W1_SCALE)
                    )
                    u_tiles.append(u_t)
                    # pair-interleaved fp8 copy for the DR head-sum matmul
                    if ci % 2 == 0:
                        u8_t = u8pool.tile(
                            [128, 2, MS], FP8, tag=f"u8_{ci // 2}", name="u8_t"
                        )
                        u8_tiles.append(u8_t)
                    nc.vector.tensor_scalar_mul(
                        u8_tiles[ci // 2][:, ci % 2, :], u_t[:], 1.0
                    )

                # ---- stripe ms-1 normalization (hides exp latency) ----
                at_tiles = emit_norm(prev_u, prev_rcp) if prev_rcp is not None else None

                # ---- stripe ms-1 output projection, first half ----
                if at_tiles is not None:
                    emit_tail(at_tiles, prev_ms, range(0, NC_ // 2))

                # ---- head sums (4-MM fp8-DR block) + reciprocal ----
                s_ps = pss.tile([HEADS, MS], F32, tag="s", name="s_ps")
                for t in range(KP):
                    nc.tensor.matmul(
                        s_ps[:],
                        sel8_t[:, t, :, :],
                        u8_tiles[t][:],
                        start=(t == 0),
                        stop=(t == KP - 1),
                        perf_mode=DR,
                    )
                rcp32 = spool.tile([HEADS, MS], F32, tag="rcp32", name="rcp32")
                nc.vector.reciprocal_approx_fast(rcp32[:], s_ps[:])
                # rcp padded to 128 partitions (rows 16+ zeroed on the idle
                # GpSimd engine) so the rb matmul runs with K=128
                rcp_t = spool.tile([128, MS], BF16, tag="rcp", name="rcp_t")
                nc.gpsimd.memset(rcp_t[:], 0.0)
                nc.scalar.copy(rcp_t[0:HEADS, :], rcp32[:])

                # ---- stripe ms-1 output projection, second half ----
                if at_tiles is not None:
                    emit_tail(at_tiles, prev_ms, range(NC_ // 2, NC_))
                prev_u, prev_rcp, prev_ms = u_tiles, rcp_t, ms

            # epilogue: last stripe's normalization + output projection
            at_tiles = emit_norm(prev_u, prev_rcp)
            emit_tail(at_tiles, prev_ms, range(NC_))
    nc.compile()
    return nc


_NC_CACHE = None
LAST_RESULT = None


def _ensure_ntff_hook():
    """bass_utils' axon trace path needs antenv.axon_hooks, which this
    container's antenv lacks. Provide it + register the ctypes NTFF hook."""
    import types

    try:
        from antenv.axon_hooks import get_axon_ntff_profile_hook  # noqa: F401
        return True
    except ImportError:
        pass
    try:
        import antenv
        from trn_agent_boot.trn_boot import _ntff_profile_via_ctypes

        m = types.ModuleType("antenv.axon_hooks")
        state = {"hook": None}
        m.set_axon_ntff_profile_hook = lambda h: state.__setitem__("hook", h)
        m.get_axon_ntff_profile_hook = lambda: state["hook"]
        sys.modules["antenv.axon_hooks"] = m
        antenv.axon_hooks = m
        m.set_axon_ntff_profile_hook(
            _ntff_profile_via_ctypes("/opt/axon/libaxon_pjrt.so")
        )
        return True
    except Exception as e:  # pragma: no cover
        print(f"ntff hook injection failed: {e}")
        return False


def _selectors():
    # head index of global feature n is n // 64; pair-chunk t group i covers
    # chunk ci = 2t+i, i.e. heads 2ci (partitions 0..63) and 2ci+1 (64..127).
    sel8 = np.zeros((128, KP, 2, HEADS), np.float32)
    for t in range(KP):
        for i in range(2):
            ci = 2 * t + i
            sel8[:64, t, i, 2 * ci] = 1.0
            sel8[64:, t, i, 2 * ci + 1] = 1.0
    selt = np.zeros((128, NC_, 128), np.float32)  # K padded to 128, rows 16+ zero
    for ci in range(NC_):
        selt[2 * ci, ci, :64] = 1.0
        selt[2 * ci + 1, ci, 64:] = 1.0
    return (
        np.ascontiguousarray(sel8.reshape(128, KP * 2 * HEADS)).astype(_F8),
        np.ascontiguousarray(selt.reshape(128, NC_ * 128)).astype(_BF),
    )


def kernel(x, W1, W2, heads, trace=False):
    global _NC_CACHE, LAST_RESULT
    x = np.asarray(x, dtype=np.float32)
    W1 = np.asarray(W1, dtype=np.float32)
    W2 = np.asarray(W2, dtype=np.float32)

    X = x.reshape(M_TOTAL, E)
    X8T = np.ascontiguousarray(X.T).astype(_F8)           # [E, M_TOTAL]
    w18 = np.ascontiguousarray((W1[:E, :] * W1_SCALE).T).astype(_F8)  # [k, n]
    w2t = np.ascontiguousarray(W2.T).astype(_BF)          # [n, j] = W2[j, n]
    sel8, selt = _selectors()

    in_maps = []
    for c in range(N_CORES):
        xt_c = np.ascontiguousarray(X8T[:, c * M_CORE:(c + 1) * M_CORE])
        in_maps.append(
            {"xt8": xt_c, "w18": w18, "w2t": w2t, "sel8": sel8, "selt": selt}
        )

    if _NC_CACHE is None:
        _NC_CACHE = build_nc()

    if trace:
        trace = _ensure_ntff_hook()

    res = run_bass_kernel_spmd(_NC_CACHE, in_maps, list(range(N_CORES)), trace=trace)
    LAST_RESULT = res

    OT = np.concatenate(
        [np.asarray(res.results[c]["outT"]).astype(np.float32) for c in range(N_CORES)],
        axis=1,
    )
    return np.ascontiguousarray(OT.T).reshape(B, S, E)


# revision 6
# speedup vs baseline: 1.3541x; 1.0072x over previous
"""Trainium2 Bass kernel for nn_Attention_9242769622327.

Math: the reference computes
    qkv = x @ W1.T ; q,k,v = split(qkv)
    score = softmax(k^T v / 4, axis=-1)            # rows sum to 1
    attn  = softmax(einsum('bhnk,bhkc->bhnk', q/4, score), axis=-1)
          = softmax(q/4 * sum_c score)             # sum_c score == 1
          = softmax(q/4)                           # k/v are mathematically dead
    out   = attn @ W2.T
so only the q-projection (first E rows of W1), a per-head (64-wide) softmax,
and the output projection are needed.

Distribution: pure data-parallel over the 32768 = B*S rows; each of the 8
cores handles 4096 rows with the full weights. No collectives.

Precision: mm1 (q-projection) and the head-sum matmul run in fp8-e4m3
DoubleRow (2 fp8 MACs per PE cell per cycle -> half the matmul instructions
of bf16; measured same 213ns/MM issue rate at N=512).  The ~2.5% fp8 noise
on q is attenuated by the /4 + exp + softmax chain to ~1% on the output; the
head-sum noise is averaged down by the 64-wide sum.  mm2 must stay fp16:
quantizing attn to fp8 alone costs ~2.5% on the output (threshold is 2%).
W1q is pre-scaled by 32 so its entries (std 1/32) use the fp8 dynamic range;
the exp() activation applies scale 1/(4*32) to compensate.

On-chip layout is fully transposed (features on partitions, rows on the free
dim) so no on-chip transposes are needed anywhere:
    qT[n,m]  = sum_k W1qT[k,n] * xT[k,m]          (PE, fp8 DR, K=256/MM)
    u        = exp(qT/128)                        (ACT, PSUM->SBUF fp16)
    u8       = fp8(u)                             (DVE, pair-interleaved)
    s[g,m]   = sum_{n in head g} u8[n,m]          (PE fp8-DR w/ 0/1 selector)
    rcp      = 1/s                                (DVE reciprocal_approx_fast)
    rb[n,m]  = rcp[head(n),m]                     (PE matmul w/ selector^T)
    aT       = u * rb                             (DVE)
    outT[j,m]= sum_n W2T[n,j] * aT[n,m]           (PE, fp16)

Stripes are software-pipelined: stripe ms runs [32 mm1-DR][8 rb(ms-1)]
[64 mm2(ms-1) with the 4 sel-DR MMs of stripe ms pushed between the mm2
j-blocks] so the exp->u8 chain never stalls the PE.  PSUM: 3 q banks, 1 s,
2 rb, 2 out.  w2/selt DMAs are deferred behind stripe-0/1 x + w1 loads so
the first mm1 block starts as early as possible.
"""

import sys

sys.path.insert(0, "/opt/trn_rl_repo")

import numpy as np
import ml_dtypes

import concourse.bass as bass
import concourse.bacc as bacc
import concourse.tile as tile
from concourse import mybir
from concourse.bass_utils import run_bass_kernel_spmd

BF16 = mybir.dt.float16  # fp16: same PE rate as bf16, 10-bit mantissa
FP8 = mybir.dt.float8e4
F32 = mybir.dt.float32
AF = mybir.ActivationFunctionType
DR = mybir.MatmulPerfMode.DoubleRow

N_CORES = 8
B, S, E = 4, 8192, 1024
HEADS, HEAD_DIM = 16, 64
M_TOTAL = B * S                # 32768
M_CORE = M_TOTAL // N_CORES    # 4096 rows per core
MS = 512                       # m-stripe width (moving free dim / PSUM bank)
N_STRIPES = M_CORE // MS       # 8
KP = E // 256                  # 4 DoubleRow contraction pair-chunks
NC_ = E // 128                 # 8 feature chunks
W1_SCALE = 32.0                # pre-scale on W1q before fp8 quantization

_BF = np.float16
_F8 = ml_dtypes.float8_e4m3fn


def build_nc() -> bass.Bass:
    nc = bacc.Bacc("TRN2", debug=False)

    xt8 = nc.dram_tensor("xt8", [E, M_CORE], FP8, kind="ExternalInput")
    w18 = nc.dram_tensor("w18", [E, E], FP8, kind="ExternalInput")
    w2t = nc.dram_tensor("w2t", [E, E], BF16, kind="ExternalInput")
    sel8 = nc.dram_tensor("sel8", [128, KP * 2 * HEADS], FP8, kind="ExternalInput")
    selt = nc.dram_tensor("selt", [128, NC_ * 128], BF16, kind="ExternalInput")
    outT = nc.dram_tensor("outT", [E, M_CORE], BF16, kind="ExternalOutput")

    # k = t*256 + two*128 + p
    x8_v = xt8[:, :].rearrange("(t two p) m -> p t two m", p=128, two=2)
    w18_v = w18[:, :].rearrange("(t two p) n -> p t two n", p=128, two=2)
    w2_v = w2t[:, :].rearrange("(c p) j -> p c j", p=128)   # [128, 8, 1024]

    with tile.TileContext(nc) as tc:
        with (
            tc.tile_pool(name="weights", bufs=1) as wpool,
            tc.tile_pool(name="xt", bufs=N_STRIPES) as xpool,
            tc.tile_pool(name="u", bufs=16) as upool,
            tc.tile_pool(name="u8", bufs=2) as u8pool,
            tc.tile_pool(name="at", bufs=16) as apool,
            tc.tile_pool(name="small", bufs=3) as spool,
            tc.tile_pool(name="ostage", bufs=8) as opool,
            tc.tile_pool(name="ps_q", bufs=3, space="PSUM") as psq,
            tc.tile_pool(name="ps_s", bufs=1, space="PSUM") as pss,
            tc.tile_pool(name="ps_rb", bufs=2, space="PSUM") as psrb,
            tc.tile_pool(name="ps_o", bufs=2, space="PSUM") as pso,
        ):
            # Warm the PE's HAM clock gate with throwaway matmuls on memset
            # scratch while the first weight/x DMAs are in flight, so the
            # first real matmuls run at 2.4 GHz instead of 1.2.  ~10 cold MMs
            # x 427ns covers the ~3.4us HAM window.
            warm_sb = wpool.tile([128, MS], BF16, name="warm_sb")
            nc.gpsimd.memset(warm_sb[:], 0.0)
            warm_ps = psq.tile([128, MS], F32, tag="q", name="warm_ps")
            for _ in range(10):
                nc.tensor.matmul(
                    warm_ps[:], warm_sb[:, 0:128], warm_sb[:], start=True, stop=True
                )

            # Stripe-0-critical loads first: w1 pair-chunks interleaved with
            # stripe-0 x pair-chunks, then the tiny sel8.  w2/selt are
            # deferred until after stripe 1's x loads (they aren't read until
            # stripe 0's normalization/output projection, ~25us in).
            w1_t = []
            xt0 = []
            for t in range(KP):
                w = wpool.tile([128, 2, E], FP8, tag=f"w1_{t}", name=f"w1t{t}")
                nc.sync.dma_start(w[:], w18_v[:, t, :, :])
                w1_t.append(w)
                tx = xpool.tile([128, 2, MS], FP8, tag=f"xt_{t}", name=f"xt0_{t}")
                nc.sync.dma_start(tx[:], x8_v[:, t, :, 0:MS])
                xt0.append(tx)
            sel8_t = wpool.tile([128, KP, 2, HEADS], FP8, name="sel8_t")
            nc.sync.dma_start(
                sel8_t[:],
                sel8[:, :].rearrange("p (t two h) -> p t two h", two=2, h=HEADS),
            )

            w2_k = [
                wpool.tile([128, E], BF16, tag=f"w2_{ci}", name=f"w2k{ci}")
                for ci in range(NC_)
            ]
            selt_t = wpool.tile([128, NC_, 128], BF16, name="selt_t")

            # Software pipeline over stripes: while stripe ms runs its
            # q-projection (mm1) + exp + head-sum on the PE, stripe ms-1's
            # normalization (rb broadcast matmul + DVE mul) and output
            # projection (mm2) are interleaved so the PE never waits on the
            # softmax chain.
            prev_u = None       # u tiles of stripe ms-1
            prev_rcp = None     # reciprocal head-sums of stripe ms-1 (fp16)
            prev_ms = -1

            def emit_norm(pu, prcp):
                """rb broadcast matmuls (PE, K padded to 128 so LDWEIGHTS
                overlaps like the main GEMM blocks) + DVE muls."""
                ats = []
                for ci in range(NC_):
                    rb_ps = psrb.tile([128, MS], F32, tag="rb", name="rb_ps")
                    nc.tensor.matmul(
                        rb_ps[:], selt_t[:, ci, :], prcp[:], start=True, stop=True
                    )
                    at_t = apool.tile([128, MS], BF16, tag="at", name="at_t")
                    nc.vector.tensor_mul(at_t[:], pu[ci][:], rb_ps[:])
                    ats.append(at_t)
                return ats

            def emit_tail(at_list, ms, js, copy_engine):
                """Emit mm2 + store for a finished stripe (at tiles ready).
                copy_engine picks which engine drains o_ps so the ACT and DVE
                FIFOs each stay ahead of the PSUM-bank rotation."""
                for j in js:
                    o_ps = pso.tile([128, MS], F32, tag="o", name="o_ps")
                    for ci in range(NC_):
                        nc.tensor.matmul(
                            o_ps[:],
                            w2_k[ci][:, j * 128:(j + 1) * 128],
                            at_list[ci][:],
                            start=(ci == 0),
                            stop=(ci == NC_ - 1),
                        )
                    o_t = opool.tile([128, MS], BF16, tag="ost", name="o_t")
                    if copy_engine == "act":
                        nc.scalar.copy(o_t[:], o_ps[:])
                    else:
                        nc.vector.tensor_scalar_mul(o_t[:], o_ps[:], 1.0)
                    nc.sync.dma_start(
                        outT[j * 128:(j + 1) * 128, ms * MS:(ms + 1) * MS], o_t[:]
                    )

            for ms in range(N_STRIPES):
                if ms == 0:
                    xt_k = xt0
                else:
                    xt_k = []
                    for t in range(KP):
                        tx = xpool.tile(
                            [128, 2, MS], FP8, tag=f"xt_{t}", name=f"xt{ms}_{t}"
                        )
                        nc.sync.dma_start(
                            tx[:], x8_v[:, t, :, ms * MS:(ms + 1) * MS]
                        )
                        xt_k.append(tx)
                if ms == 1:
                    # deferred bulk loads (needed from stripe-0 norm onwards)
                    nc.sync.dma_start(
                        selt_t[:],
                        selt[:, :].rearrange("p (c q) -> p c q", q=128),
                    )
                    for ci in range(NC_):
                        nc.sync.dma_start(w2_k[ci][:], w2_v[:, ci, :])

                # ---- mm1: q-projection, contiguous 32-MM fp8-DR block ----
                u_tiles = []
                for ci in range(NC_):
                    q_ps = psq.tile([128, MS], F32, tag="q", name="q_ps")
                    for t in range(KP):
                        nc.tensor.matmul(
                            q_ps[:],
                            w1_t[t][:, :, ci * 128:(ci + 1) * 128],
                            xt_k[t][:],
                            start=(t == 0),
                            stop=(t == KP - 1),
                            perf_mode=DR,
                        )
                    u_t = upool.tile([128, MS], BF16, tag="u", name="u_t")
                    nc.scalar.activation(
                        u_t[:], q_ps[:], AF.Exp, scale=1.0 / (4.0 * W1_SCALE)
                    )
                    u_tiles.append(u_t)

                # ---- stripe ms-1 normalization (hides exp latency).
                # Emitted BEFORE the u8 conversions: the DVE queue is FIFO,
                # and mm2 blocks on the at tiles, while the sel head-sum (the
                # u8 consumer) runs ~8us later. ----
                at_tiles = emit_norm(prev_u, prev_rcp) if prev_rcp is not None else None

                # pair-interleaved fp8 copies for the DR head-sum matmul
                u8_tiles = []
                for ci in range(NC_):
                    if ci % 2 == 0:
                        u8_t = u8pool.tile(
                            [128, 2, MS], FP8, tag=f"u8_{ci // 2}", name="u8_t"
                        )
                        u8_tiles.append(u8_t)
                    nc.vector.tensor_scalar_mul(
                        u8_tiles[ci // 2][:, ci % 2, :], u_tiles[ci][:], 1.0
                    )

                # ---- stripe ms-1 output projection, first half ----
                if at_tiles is not None:
                    emit_tail(at_tiles, prev_ms, range(0, NC_ // 2), "act")

                # ---- head sums (4-MM fp8-DR block) + reciprocal ----
                s_ps = pss.tile([HEADS, MS], F32, tag="s", name="s_ps")
                for t in range(KP):
                    nc.tensor.matmul(
                        s_ps[:],
                        sel8_t[:, t, :, :],
                        u8_tiles[t][:],
                        start=(t == 0),
                        stop=(t == KP - 1),
                        perf_mode=DR,
                    )
                rcp32 = spool.tile([HEADS, MS], F32, tag="rcp32", name="rcp32")
                nc.vector.reciprocal_approx_fast(rcp32[:], s_ps[:])
                # rcp padded to 128 partitions (rows 16+ zeroed on the idle
                # GpSimd engine) so the rb matmul runs with K=128
                rcp_t = spool.tile([128, MS], BF16, tag="rcp", name="rcp_t")
                nc.gpsimd.memset(rcp_t[:], 0.0)
                nc.scalar.copy(rcp_t[0:HEADS, :], rcp32[:])

                # ---- stripe ms-1 output projection, second half ----
                if at_tiles is not None:
                    emit_tail(at_tiles, prev_ms, range(NC_ // 2, NC_), "dve")
                prev_u, prev_rcp, prev_ms = u_tiles, rcp_t, ms

            # epilogue: last stripe's normalization + output projection
            at_tiles = emit_norm(prev_u, prev_rcp)
            emit_tail(at_tiles, prev_ms, range(0, NC_ // 2), "act")
            emit_tail(at_tiles, prev_ms, range(NC_ // 2, NC_), "dve")
    nc.compile()
    return nc


_NC_CACHE = None
LAST_RESULT = None


def _ensure_ntff_hook():
    """bass_utils' axon trace path needs antenv.axon_hooks, which this
    container's antenv lacks. Provide it + register the ctypes NTFF hook."""
    import types

    try:
        from antenv.axon_hooks import get_axon_ntff_profile_hook  # noqa: F401
        return True
    except ImportError:
        pass
    try:
        import antenv
        from trn_agent_boot.trn_boot import _ntff_profile_via_ctypes

        m = types.ModuleType("antenv.axon_hooks")
        state = {"hook": None}
        m.set_axon_ntff_profile_hook = lambda h: state.__setitem__("hook", h)
        m.get_axon_ntff_profile_hook = lambda: state["hook"]
        sys.modules["antenv.axon_hooks"] = m
        antenv.axon_hooks = m
        m.set_axon_ntff_profile_hook(
            _ntff_profile_via_ctypes("/opt/axon/libaxon_pjrt.so")
        )
        return True
    except Exception as e:  # pragma: no cover
        print(f"ntff hook injection failed: {e}")
        return False


def _selectors():
    # head index of global feature n is n // 64; pair-chunk t group i covers
    # chunk ci = 2t+i, i.e. heads 2ci (partitions 0..63) and 2ci+1 (64..127).
    sel8 = np.zeros((128, KP, 2, HEADS), np.float32)
    for t in range(KP):
        for i in range(2):
            ci = 2 * t + i
            sel8[:64, t, i, 2 * ci] = 1.0
            sel8[64:, t, i, 2 * ci + 1] = 1.0
    selt = np.zeros((128, NC_, 128), np.float32)  # K padded to 128, rows 16+ zero
    for ci in range(NC_):
        selt[2 * ci, ci, :64] = 1.0
        selt[2 * ci + 1, ci, 64:] = 1.0
    return (
        np.ascontiguousarray(sel8.reshape(128, KP * 2 * HEADS)).astype(_F8),
        np.ascontiguousarray(selt.reshape(128, NC_ * 128)).astype(_BF),
    )


def kernel(x, W1, W2, heads, trace=False):
    global _NC_CACHE, LAST_RESULT
    x = np.asarray(x, dtype=np.float32)
    W1 = np.asarray(W1, dtype=np.float32)
    W2 = np.asarray(W2, dtype=np.float32)

    X = x.reshape(M_TOTAL, E)
    X8T = np.ascontiguousarray(X.T).astype(_F8)           # [E, M_TOTAL]
    w18 = np.ascontiguousarray((W1[:E, :] * W1_SCALE).T).astype(_F8)  # [k, n]
    w2t = np.ascontiguousarray(W2.T).astype(_BF)          # [n, j] = W2[j, n]
    sel8, selt = _selectors()

    in_maps = []
    for c in range(N_CORES):
        xt_c = np.ascontiguousarray(X8T[:, c * M_CORE:(c + 1) * M_CORE])
        in_maps.append(
            {"xt8": xt_c, "w18": w18, "w2t": w2t, "sel8": sel8, "selt": selt}
        )

    if _NC_CACHE is None:
        _NC_CACHE = build_nc()

    if trace:
        trace = _ensure_ntff_hook()

    res = run_bass_kernel_spmd(_NC_CACHE, in_maps, list(range(N_CORES)), trace=trace)
    LAST_RESULT = res

    OT = np.concatenate(
        [np.asarray(res.results[c]["outT"]).astype(np.float32) for c in range(N_CORES)],
        axis=1,
    )
    return np.ascontiguousarray(OT.T).reshape(B, S, E)


# revision 9
# speedup vs baseline: 1.3570x; 1.0021x over previous
"""Trainium2 Bass kernel for nn_Attention_9242769622327.

Math: the reference computes
    qkv = x @ W1.T ; q,k,v = split(qkv)
    score = softmax(k^T v / 4, axis=-1)            # rows sum to 1
    attn  = softmax(einsum('bhnk,bhkc->bhnk', q/4, score), axis=-1)
          = softmax(q/4 * sum_c score)             # sum_c score == 1
          = softmax(q/4)                           # k/v are mathematically dead
    out   = attn @ W2.T
so only the q-projection (first E rows of W1), a per-head (64-wide) softmax,
and the output projection are needed.

Distribution: pure data-parallel over the 32768 = B*S rows; each of the 8
cores handles 4096 rows with the full weights. No collectives.

Precision: mm1 (q-projection) and the head-sum matmul run in fp8-e4m3
DoubleRow (2 fp8 MACs per PE cell per cycle -> half the matmul instructions
of bf16; measured same 213ns/MM issue rate at N=512).  The ~2.5% fp8 noise
on q is attenuated by the /4 + exp + softmax chain to ~1% on the output; the
head-sum noise is averaged down by the 64-wide sum.  mm2 must stay fp16:
quantizing attn to fp8 alone costs ~2.5% on the output (threshold is 2%).
W1q is pre-scaled by 32 so its entries (std 1/32) use the fp8 dynamic range;
the exp() activation applies scale 1/(4*32) to compensate.

On-chip layout is fully transposed (features on partitions, rows on the free
dim) so no on-chip transposes are needed anywhere:
    qT[n,m]  = sum_k W1qT[k,n] * xT[k,m]          (PE, fp8 DR, K=256/MM)
    u        = exp(qT/128)                        (ACT, PSUM->SBUF fp16)
    u8       = fp8(u)                             (DVE, pair-interleaved)
    s[g,m]   = sum_{n in head g} u8[n,m]          (PE fp8-DR w/ 0/1 selector)
    rcp      = 1/s                                (DVE reciprocal_approx_fast)
    rb[n,m]  = rcp[head(n),m]                     (PE matmul w/ selector^T)
    aT       = u * rb                             (DVE)
    outT[j,m]= sum_n W2T[n,j] * aT[n,m]           (PE, fp16)

Stripes are software-pipelined: stripe ms runs [32 mm1-DR][8 rb(ms-1)]
[64 mm2(ms-1) with the 4 sel-DR MMs of stripe ms pushed between the mm2
j-blocks] so the exp->u8 chain never stalls the PE.  PSUM: 3 q banks, 1 s,
2 rb, 2 out.  w2/selt DMAs are deferred behind stripe-0/1 x + w1 loads so
the first mm1 block starts as early as possible.
"""

import sys

sys.path.insert(0, "/opt/trn_rl_repo")

import numpy as np
import ml_dtypes

import concourse.bass as bass
import concourse.bacc as bacc
import concourse.tile as tile
from concourse import mybir
from concourse.bass_utils import run_bass_kernel_spmd

BF16 = mybir.dt.float16  # fp16: same PE rate as bf16, 10-bit mantissa
FP8 = mybir.dt.float8e4
F32 = mybir.dt.float32
AF = mybir.ActivationFunctionType
DR = mybir.MatmulPerfMode.DoubleRow

N_CORES = 8
B, S, E = 4, 8192, 1024
HEADS, HEAD_DIM = 16, 64
M_TOTAL = B * S                # 32768
M_CORE = M_TOTAL // N_CORES    # 4096 rows per core
MS = 512                       # m-stripe width (moving free dim / PSUM bank)
N_STRIPES = M_CORE // MS       # 8
KP = E // 256                  # 4 DoubleRow contraction pair-chunks
NC_ = E // 128                 # 8 feature chunks
W1_SCALE = 32.0                # pre-scale on W1q before fp8 quantization

_BF = np.float16
_F8 = ml_dtypes.float8_e4m3fn


def build_nc() -> bass.Bass:
    nc = bacc.Bacc("TRN2", debug=False)

    xt8 = nc.dram_tensor("xt8", [E, M_CORE], FP8, kind="ExternalInput")
    w18 = nc.dram_tensor("w18", [E, E], FP8, kind="ExternalInput")
    w2t = nc.dram_tensor("w2t", [E, E], BF16, kind="ExternalInput")
    sel8 = nc.dram_tensor("sel8", [128, KP * 2 * HEADS], FP8, kind="ExternalInput")
    selt = nc.dram_tensor("selt", [128, NC_ * 128], BF16, kind="ExternalInput")
    outT = nc.dram_tensor("outT", [E, M_CORE], BF16, kind="ExternalOutput")

    # k = t*256 + two*128 + p
    x8_v = xt8[:, :].rearrange("(t two p) m -> p t two m", p=128, two=2)
    w18_v = w18[:, :].rearrange("(t two p) n -> p t two n", p=128, two=2)
    w2_v = w2t[:, :].rearrange("(c p) j -> p c j", p=128)   # [128, 8, 1024]

    with tile.TileContext(nc) as tc:
        with (
            tc.tile_pool(name="weights", bufs=1) as wpool,
            tc.tile_pool(name="xt", bufs=N_STRIPES) as xpool,
            tc.tile_pool(name="u", bufs=24) as upool,
            tc.tile_pool(name="u8", bufs=2) as u8pool,
            tc.tile_pool(name="at", bufs=16) as apool,
            tc.tile_pool(name="small", bufs=3) as spool,
            tc.tile_pool(name="ostage", bufs=8) as opool,
            tc.tile_pool(name="ps_q", bufs=3, space="PSUM") as psq,
            tc.tile_pool(name="ps_s", bufs=1, space="PSUM") as pss,
            tc.tile_pool(name="ps_rb", bufs=2, space="PSUM") as psrb,
            tc.tile_pool(name="ps_o", bufs=2, space="PSUM") as pso,
        ):
            # Warm the PE's HAM clock gate with throwaway matmuls on memset
            # scratch while the first weight/x DMAs are in flight, so the
            # first real matmuls run at 2.4 GHz instead of 1.2.  ~10 cold MMs
            # x 427ns covers the ~3.4us HAM window.
            warm_sb = wpool.tile([128, MS], BF16, name="warm_sb")
            nc.gpsimd.memset(warm_sb[:], 0.0)
            warm_ps = psq.tile([128, MS], F32, tag="q", name="warm_ps")
            for _ in range(10):
                nc.tensor.matmul(
                    warm_ps[:], warm_sb[:, 0:128], warm_sb[:], start=True, stop=True
                )

            # Stripe-0-critical loads first: w1 pair-chunks interleaved with
            # stripe-0 x pair-chunks, then the tiny sel8.  w2/selt are
            # deferred until after stripe 1's x loads (they aren't read until
            # stripe 0's normalization/output projection, ~25us in).
            w1_t = []
            xt0 = []
            for t in range(KP):
                w = wpool.tile([128, 2, E], FP8, tag=f"w1_{t}", name=f"w1t{t}")
                nc.sync.dma_start(w[:], w18_v[:, t, :, :])
                w1_t.append(w)
                tx = xpool.tile([128, 2, MS], FP8, tag=f"xt_{t}", name=f"xt0_{t}")
                nc.sync.dma_start(tx[:], x8_v[:, t, :, 0:MS])
                xt0.append(tx)
            sel8_t = wpool.tile([128, KP, 2, HEADS], FP8, name="sel8_t")
            nc.sync.dma_start(
                sel8_t[:],
                sel8[:, :].rearrange("p (t two h) -> p t two h", two=2, h=HEADS),
            )

            w2_k = [
                wpool.tile([128, E], BF16, tag=f"w2_{ci}", name=f"w2k{ci}")
                for ci in range(NC_)
            ]
            selt_t = wpool.tile([128, NC_, 128], BF16, name="selt_t")

            # Software pipeline over stripes, depth 3, so each iteration has
            # exactly ONE fp8-DR matmul block [mm1(ms), sel(ms-1)] and ONE
            # fp16 block [rb(ms-2), mm2(ms-2)].  The PE pays ~1 extra MM slot
            # per DR<->fp16 mode switch (the incoming mode's first LDWEIGHTS
            # cannot overlap the outgoing mode's matmul), so transitions are
            # consolidated to 2 per stripe.
            state = {}          # ms -> dict(u=, u8=, rcp=, at=)

            def emit_norm(pu, prcp):
                """rb broadcast matmuls (PE, K padded to 128 so LDWEIGHTS
                overlaps like the main GEMM blocks) + DVE muls."""
                ats = []
                for ci in range(NC_):
                    rb_ps = psrb.tile([128, MS], F32, tag="rb", name="rb_ps")
                    nc.tensor.matmul(
                        rb_ps[:], selt_t[:, ci, :], prcp[:], start=True, stop=True
                    )
                    at_t = apool.tile([128, MS], BF16, tag="at", name="at_t")
                    nc.vector.tensor_mul(at_t[:], pu[ci][:], rb_ps[:])
                    ats.append(at_t)
                return ats

            def emit_tail(at_list, ms, js, copy_engine):
                """Emit mm2 + store for a finished stripe (at tiles ready).
                copy_engine picks which engine drains o_ps so the ACT and DVE
                FIFOs each stay ahead of the PSUM-bank rotation."""
                for j in js:
                    o_ps = pso.tile([128, MS], F32, tag="o", name="o_ps")
                    for ci in range(NC_):
                        nc.tensor.matmul(
                            o_ps[:],
                            w2_k[ci][:, j * 128:(j + 1) * 128],
                            at_list[ci][:],
                            start=(ci == 0),
                            stop=(ci == NC_ - 1),
                        )
                    o_t = opool.tile([128, MS], BF16, tag="ost", name="o_t")
                    if copy_engine == "act":
                        nc.scalar.copy(o_t[:], o_ps[:])
                    else:
                        nc.vector.tensor_scalar_mul(o_t[:], o_ps[:], 1.0)
                    nc.sync.dma_start(
                        outT[j * 128:(j + 1) * 128, ms * MS:(ms + 1) * MS], o_t[:]
                    )

            for ms in range(N_STRIPES):
                if ms == 0:
                    xt_k = xt0
                else:
                    xt_k = []
                    for t in range(KP):
                        tx = xpool.tile(
                            [128, 2, MS], FP8, tag=f"xt_{t}", name=f"xt{ms}_{t}"
                        )
                        nc.sync.dma_start(
                            tx[:], x8_v[:, t, :, ms * MS:(ms + 1) * MS]
                        )
                        xt_k.append(tx)
                if ms == 1:
                    # deferred bulk loads (needed from stripe-0 norm onwards)
                    nc.sync.dma_start(
                        selt_t[:],
                        selt[:, :].rearrange("p (c q) -> p c q", q=128),
                    )
                    for ci in range(NC_):
                        nc.sync.dma_start(w2_k[ci][:], w2_v[:, ci, :])

                # ---- DR block: mm1(ms), then head-sum of stripe ms-1 ----
                u_tiles = []
                for ci in range(NC_):
                    q_ps = psq.tile([128, MS], F32, tag="q", name="q_ps")
                    for t in range(KP):
                        nc.tensor.matmul(
                            q_ps[:],
                            w1_t[t][:, :, ci * 128:(ci + 1) * 128],
                            xt_k[t][:],
                            start=(t == 0),
                            stop=(t == KP - 1),
                            perf_mode=DR,
                        )
                    u_t = upool.tile([128, MS], BF16, tag="u", name="u_t")
                    nc.scalar.activation(
                        u_t[:], q_ps[:], AF.Exp, scale=1.0 / (4.0 * W1_SCALE)
                    )
                    u_tiles.append(u_t)
                state[ms] = {"u": u_tiles}

                if ms - 1 in state:
                    p1 = state[ms - 1]
                    s_ps = pss.tile([HEADS, MS], F32, tag="s", name="s_ps")
                    for t in range(KP):
                        nc.tensor.matmul(
                            s_ps[:],
                            sel8_t[:, t, :, :],
                            p1["u8"][t][:],
                            start=(t == 0),
                            stop=(t == KP - 1),
                            perf_mode=DR,
                        )
                    rcp32 = spool.tile([HEADS, MS], F32, tag="rcp32", name="rcp32")
                    nc.vector.reciprocal_approx_fast(rcp32[:], s_ps[:])
                    # rcp padded to 128 partitions (rows 16+ zeroed on the
                    # idle GpSimd engine) so the rb matmul runs with K=128
                    rcp_t = spool.tile([128, MS], BF16, tag="rcp", name="rcp_t")
                    nc.gpsimd.memset(rcp_t[:], 0.0)
                    nc.scalar.copy(rcp_t[0:HEADS, :], rcp32[:])
                    p1["rcp"] = rcp_t

                # ---- fp16 block: normalization + output proj of ms-2 ----
                # (at-muls precede the u8 conversions in the DVE FIFO: mm2
                # blocks on at tiles; sel's u8 isn't read until next stripe)
                if ms - 2 in state:
                    p2 = state.pop(ms - 2)
                    at_tiles = emit_norm(p2["u"], p2["rcp"])
                    emit_tail(at_tiles, ms - 2, range(0, NC_ // 2), "act")
                    emit_tail(at_tiles, ms - 2, range(NC_ // 2, NC_), "dve")

                # pair-interleaved fp8 copies of u(ms) for next stripe's
                # DR head-sum matmul (late in the DVE FIFO on purpose)
                u8_tiles = []
                for ci in range(NC_):
                    if ci % 2 == 0:
                        u8_t = u8pool.tile(
                            [128, 2, MS], FP8, tag=f"u8_{ci // 2}", name="u8_t"
                        )
                        u8_tiles.append(u8_t)
                    nc.vector.tensor_scalar_mul(
                        u8_tiles[ci // 2][:, ci % 2, :], u_tiles[ci][:], 1.0
                    )
                state[ms]["u8"] = u8_tiles

            # epilogue: head-sum of stripe 7, then the two remaining
            # normalization + output projection stages (one DR<->fp16 switch)
            last = N_STRIPES - 1
            p1 = state[last]
            s_ps = pss.tile([HEADS, MS], F32, tag="s", name="s_ps")
            for t in range(KP):
                nc.tensor.matmul(
                    s_ps[:],
                    sel8_t[:, t, :, :],
                    p1["u8"][t][:],
                    start=(t == 0),
                    stop=(t == KP - 1),
                    perf_mode=DR,
                )
            rcp32 = spool.tile([HEADS, MS], F32, tag="rcp32", name="rcp32")
            nc.vector.reciprocal_approx_fast(rcp32[:], s_ps[:])
            rcp_t = spool.tile([128, MS], BF16, tag="rcp", name="rcp_t")
            nc.gpsimd.memset(rcp_t[:], 0.0)
            nc.scalar.copy(rcp_t[0:HEADS, :], rcp32[:])
            p1["rcp"] = rcp_t

            for ms in (N_STRIPES - 2, N_STRIPES - 1):
                p = state.pop(ms)
                at_tiles = emit_norm(p["u"], p["rcp"])
                emit_tail(at_tiles, ms, range(0, NC_ // 2), "act")
                emit_tail(at_tiles, ms, range(NC_ // 2, NC_), "dve")
    nc.compile()
    return nc


_NC_CACHE = None
LAST_RESULT = None


def _ensure_ntff_hook():
    """bass_utils' axon trace path needs antenv.axon_hooks, which this
    container's antenv lacks. Provide it + register the ctypes NTFF hook."""
    import types

    try:
        from antenv.axon_hooks import get_axon_ntff_profile_hook  # noqa: F401
        return True
    except ImportError:
        pass
    try:
        import antenv
        from trn_agent_boot.trn_boot import _ntff_profile_via_ctypes

        m = types.ModuleType("antenv.axon_hooks")
        state = {"hook": None}
        m.set_axon_ntff_profile_hook = lambda h: state.__setitem__("hook", h)
        m.get_axon_ntff_profile_hook = lambda: state["hook"]
        sys.modules["antenv.axon_hooks"] = m
        antenv.axon_hooks = m
        m.set_axon_ntff_profile_hook(
            _ntff_profile_via_ctypes("/opt/axon/libaxon_pjrt.so")
        )
        return True
    except Exception as e:  # pragma: no cover
        print(f"ntff hook injection failed: {e}")
        return False


def _selectors():
    # head index of global feature n is n // 64; pair-chunk t group i covers
    # chunk ci = 2t+i, i.e. heads 2ci (partitions 0..63) and 2ci+1 (64..127).
    sel8 = np.zeros((128, KP, 2, HEADS), np.float32)
    for t in range(KP):
        for i in range(2):
            ci = 2 * t + i
            sel8[:64, t, i, 2 * ci] = 1.0
            sel8[64:, t, i, 2 * ci + 1] = 1.0
    selt = np.zeros((128, NC_, 128), np.float32)  # K padded to 128, rows 16+ zero
    for ci in range(NC_):
        selt[2 * ci, ci, :64] = 1.0
        selt[2 * ci + 1, ci, 64:] = 1.0
    return (
        np.ascontiguousarray(sel8.reshape(128, KP * 2 * HEADS)).astype(_F8),
        np.ascontiguousarray(selt.reshape(128, NC_ * 128)).astype(_BF),
    )


def kernel(x, W1, W2, heads, trace=False):
    global _NC_CACHE, LAST_RESULT
    x = np.asarray(x, dtype=np.float32)
    W1 = np.asarray(W1, dtype=np.float32)
    W2 = np.asarray(W2, dtype=np.float32)

    X = x.reshape(M_TOTAL, E)
    X8T = np.ascontiguousarray(X.T).astype(_F8)           # [E, M_TOTAL]
    w18 = np.ascontiguousarray((W1[:E, :] * W1_SCALE).T).astype(_F8)  # [k, n]
    w2t = np.ascontiguousarray(W2.T).astype(_BF)          # [n, j] = W2[j, n]
    sel8, selt = _selectors()

    in_maps = []
    for c in range(N_CORES):
        xt_c = np.ascontiguousarray(X8T[:, c * M_CORE:(c + 1) * M_CORE])
        in_maps.append(
            {"xt8": xt_c, "w18": w18, "w2t": w2t, "sel8": sel8, "selt": selt}
        )

    if _NC_CACHE is None:
        _NC_CACHE = build_nc()

    if trace:
        trace = _ensure_ntff_hook()

    res = run_bass_kernel_spmd(_NC_CACHE, in_maps, list(range(N_CORES)), trace=trace)
    LAST_RESULT = res

    OT = np.concatenate(
        [np.asarray(res.results[c]["outT"]).astype(np.float32) for c in range(N_CORES)],
        axis=1,
    )
    return np.ascontiguousarray(OT.T).reshape(B, S, E)
